# revision 9
# baseline (speedup 1.0000x reference)
"""Trainium2 Bass kernel: 4096x4096 valid 5x5 cross-correlation + scalar bias.

Strategy (8 NeuronCores, SPMD):
  - Shard the OUTPUT by columns: core c computes out[:, 512c : 512c+512]
    (core 7's last 4 columns are padding, trimmed after gather). Each core
    reads x rows 0..4095, cols [512c, 512c+516) (host-padded to width 4100).
  - On-core: the 5x5 conv is computed as banded-matrix matmuls on the
    TensorEngine. For an input row-tile X_g = x[124g : 124g+128, :] and
    kernel column dj, the banded matrix B_dj[k, m] = w[k-m, dj] gives
      (B_dj^T @ X_g[:, dj:dj+512])[m, n] = sum_di w[di, dj] x[124g+m+di, n+dj]
    so accumulating the dj-matmuls in PSUM yields 124 valid output rows
    per tile. 4092 = 33 * 124 exactly; 33 tiles cover rows 0..4095 exactly.
  - fp8 DoubleRow: operands are float8e4 (e4m3) and matmuls run in
    MatmulPerfMode.DoubleRow, which computes W0^T@X0 + W1^T@X1 per pass at
    0.5 cycles/row — two banded taps per 256-cycle matmul instead of one
    per 512. To stay inside the 2e-2 error gate, x and w are split into
    e4m3 hi+lo parts (x = xh+xl, w = wh+wl) and the three significant
    cross terms are kept: wh*xh, wl*xh, wh*xl (15 banded products + 1 zero
    slot = 8 DoubleRow matmuls per tile; measured rel err ~2.1e-3, same
    as plain bf16). PE time per tile: 8*256 cycles vs bf16's 5*512.
  - Input is pre-tiled on the host into partition-major layout
    (128, 33*1032) fp8: partition p, tile g holds [xh | xl] of x row
    124g+p, and streams as a few large DMAs (fewer descriptor pushes,
    full 16-engine fan-out).
  - Output is staged in SBUF as bf16 (PSUM->SBUF drain fuses the bias add
    and the f32->bf16 cast, split across DVE/ACT engines) and leaves the
    device partition-major as (128, 33*512) bf16 in 9 per-block DMAs.
    The host transposes back to row-major and upcasts to f32.
"""
import os

os.environ.setdefault("MYCRO_LOCAL_CACHE", "1")

import numpy as np
import ml_dtypes
import bass_rust

import concourse.bass as bass
import concourse.bacc as bacc
import concourse.tile as tile
import concourse.mybir as mybir
from concourse import bass_utils

H, W = 4096, 4096
KH, KW = 5, 5
OH, OW = H - KH + 1, W - KW + 1          # 4092, 4092
NCORES = 8
COLS = 512                               # output cols per core
XC = COLS + KW - 1                       # 516 input cols per core
XT = 2 * XC                              # 1032: [xh | xl] per tile
NG = 33                                  # row tiles per core (33*124 = 4092)
RV = 124                                 # valid output rows per tile
BLK = 4                                  # tiles per output-DMA block
# input stream chunk sizes (tiles per DMA): small first chunk so the first
# matmul starts early, larger after to amortize descriptor-push cost
CHUNKS = [1, 2, 3, 4, 5, 6, 6, 6]
assert sum(CHUNKS) == NG

# DoubleRow pair schedule: ((wpart_a, dj_a, xpart_a), (wpart_b, dj_b, xpart_b))
# wpart: 0=wh 1=wl 2=zero; xpart: 0=xh 1=xl. Within a pair both ifmap slices
# must come from the same x tile (arbitrary element delta is fine).
PAIRS = [
    ((0, 0, 0), (0, 1, 0)),   # wh*xh dj 0,1
    ((0, 2, 0), (0, 3, 0)),   # wh*xh dj 2,3
    ((1, 0, 0), (0, 4, 0)),   # wl*xh dj0, wh*xh dj4  (delta +4)
    ((1, 1, 0), (1, 2, 0)),   # wl*xh dj 1,2
    ((1, 3, 0), (1, 4, 0)),   # wl*xh dj 3,4
    ((0, 0, 1), (0, 1, 1)),   # wh*xl dj 0,1
    ((0, 2, 1), (0, 3, 1)),   # wh*xl dj 2,3
    ((0, 4, 1), (2, 4, 1)),   # wh*xl dj4, zero slot
]
NP_ = len(PAIRS)                         # 8 DoubleRow matmuls per tile

_compiled = None
TRACE = False            # test harness can flip this for neuron-profile timing
LAST_EXEC_NS = None

FP8 = mybir.dt.float8e4
BF16 = mybir.dt.bfloat16
E4 = ml_dtypes.float8_e4m3


def _build():
    nc = bacc.Bacc("TRN2", target_bir_lowering=False, debug=False,
                   num_devices=NCORES)

    x_dram = nc.dram_tensor("xs", (128, NG * XT), FP8, kind="ExternalInput")
    b_dram = nc.dram_tensor("bmat", (128, NP_ * 2 * 128), FP8,
                            kind="ExternalInput")
    bias_dram = nc.dram_tensor("biast", (128, 1), mybir.dt.float32,
                               kind="ExternalInput")
    out_dram = nc.dram_tensor("out", (128, NG * COLS), BF16,
                              kind="ExternalOutput")

    cstart = [0]
    for c in CHUNKS:
        cstart.append(cstart[-1] + c)
    blocks = [list(range(s, min(s + BLK, NG))) for s in range(0, NG, BLK)]

    def pair_ap(base2d, delta):
        """[128, 2, 512] ifmap AP: slice pair at (offset, offset+delta)."""
        ap = [list(base2d.ap[0]), [delta, 2], list(base2d.ap[1])]
        return bass_rust.AP(tensor=base2d.tensor, ap=ap,
                            offset=base2d.offset)

    def w_ap(bt2d, pi):
        """[128, 2, 128] stationary AP for pair pi."""
        ap = [list(bt2d.ap[0]), [128, 2], [1, 128]]
        return bass_rust.AP(tensor=bt2d.tensor, ap=ap,
                            offset=bt2d.offset + pi * 256)

    with tile.TileContext(nc) as tc:
        with (
            tc.tile_pool(name="const", bufs=1) as cpool,
            tc.tile_pool(name="x", bufs=1) as xpool,
            tc.tile_pool(name="stage", bufs=4) as spool,
            tc.tile_pool(name="psum", bufs=8, space=bass.MemorySpace.PSUM) as ppool,
        ):
            bt = cpool.tile([128, NP_ * 2 * 128], FP8)
            biast = cpool.tile([128, 1], mybir.dt.float32)

            xts = [xpool.tile([128, csz * XT], FP8, name=f"xc{ci}",
                              tag=f"xc{ci}")
                   for ci, csz in enumerate(CHUNKS)]

            def push_chunk(ci, eng):
                eng.dma_start(
                    xts[ci][:],
                    x_dram.ap()[:, cstart[ci] * XT:cstart[ci + 1] * XT])

            # bt gates the first matmul: put it first on the sync ring,
            # interleave early chunks across both HWDGE rings
            nc.sync.dma_start(bt[:], b_dram.ap())
            push_chunk(0, nc.sync)
            push_chunk(1, nc.scalar)
            push_chunk(2, nc.sync)
            push_chunk(3, nc.scalar)
            nc.scalar.dma_start(biast[:], bias_dram.ap())
            for ci in range(4, len(CHUNKS)):
                push_chunk(ci, nc.sync if ci % 2 == 0 else nc.scalar)

            g2chunk = []
            for ci, csz in enumerate(CHUNKS):
                g2chunk += [ci] * csz

            # gpsimd cannot read PSUM; alternate DVE / ACT for the drains
            drain_eng = lambda lg: [nc.vector, nc.scalar,
                                    nc.vector, nc.scalar][lg % 4]
            # gpsimd's SWDGE queue is slow; keep the two tail blocks on the
            # fast HWDGE rings so the kernel ends promptly
            out_ring = lambda bi: [nc.gpsimd, nc.scalar, nc.sync][bi % 3] \
                if bi < len(blocks) - 2 else (nc.scalar, nc.sync)[len(blocks) - 1 - bi]

            for bi, blk in enumerate(blocks):
                stg = spool.tile([128, len(blk) * COLS], BF16,
                                 name=f"stg{bi}", tag="stg")
                for lg, g in enumerate(blk):
                    ci = g2chunk[g]
                    toff = (g - cstart[ci]) * XT
                    xt = xts[ci]
                    ps = ppool.tile([128, COLS], mybir.dt.float32,
                                    name=f"ps{g}", tag="ps")
                    for pi, (a, b) in enumerate(PAIRS):
                        oa = toff + a[2] * XC + a[1]
                        ob = toff + b[2] * XC + b[1]
                        rhs = pair_ap(xt[:, oa:oa + COLS], ob - oa)
                        nc.tensor.matmul(
                            ps[:],
                            w_ap(bt[:], pi),
                            rhs,
                            start=(pi == 0),
                            stop=(pi == NP_ - 1),
                            perf_mode=mybir.MatmulPerfMode.DoubleRow,
                        )
                    # drain PSUM -> stage (bias add + f32->bf16 cast); all
                    # 128 rows so the staged tile is fully defined (rows
                    # 124..127 are partial sums the host discards)
                    dst = stg[:, lg * COLS:(lg + 1) * COLS]
                    if g == NG - 1:
                        # final tile: split the drain across both engines so
                        # the critical tail (drain -> last DMA -> done) is
                        # as short as possible
                        nc.vector.tensor_scalar_add(
                            dst[:, :COLS // 2], ps[:, :COLS // 2], biast[:])
                        nc.scalar.activation(
                            dst[:, COLS // 2:], ps[:, COLS // 2:],
                            mybir.ActivationFunctionType.Identity,
                            bias=biast[:])
                    elif drain_eng(lg) is nc.scalar:
                        nc.scalar.activation(
                            dst, ps[:],
                            mybir.ActivationFunctionType.Identity,
                            bias=biast[:])
                    else:
                        nc.vector.tensor_scalar_add(dst, ps[:], biast[:])
                # one output DMA per block: contiguous partition-major span
                out_ring(bi).dma_start(
                    out_dram.ap()[:, blk[0] * COLS:(blk[-1] + 1) * COLS],
                    stg[:, :len(blk) * COLS])

    nc.compile()
    return nc


def _banded(wcol: np.ndarray) -> np.ndarray:
    """128x128 banded matrix B[k, m] = wcol[k - m]."""
    b = np.zeros((128, 128), dtype=np.float32)
    for di in range(KH):
        m = np.arange(128 - di)
        b[m + di, m] = wcol[di]
    return b


def kernel(x: np.ndarray, weight: np.ndarray, bias: np.ndarray) -> np.ndarray:
    global _compiled
    x = np.ascontiguousarray(np.asarray(x, dtype=np.float32))
    weight = np.asarray(weight, dtype=np.float32)
    bias = np.asarray(bias, dtype=np.float32)

    if _compiled is None:
        _compiled = _build()
    nc = _compiled

    xpad = np.zeros((H, NCORES * COLS + KW - 1), dtype=np.float32)
    xpad[:, :W] = x
    xh8 = xpad.astype(E4)
    xl8 = (xpad - xh8.astype(np.float32)).astype(E4)

    wh = weight.astype(E4).astype(np.float32)
    wl = (weight - wh).astype(E4).astype(np.float32)
    wparts = [wh, wl, np.zeros_like(wh)]
    bmat = np.zeros((128, NP_ * 2 * 128), dtype=np.float32)
    for pi, pair in enumerate(PAIRS):
        for j, (wp, dj, _xp) in enumerate(pair):
            c0 = (2 * pi + j) * 128
            bmat[:, c0:c0 + 128] = _banded(wparts[wp][:, dj])
    bmat = bmat.astype(E4)
    bias_col = np.full((128, 1), bias[0], dtype=np.float32)

    in_maps = []
    s0, s1 = xh8.strides
    for c in range(NCORES):
        xt = np.empty((128, NG, 2, XC), dtype=E4)
        for part, src in ((0, xh8), (1, xl8)):
            sub = src[:, COLS * c: COLS * c + XC]
            win = np.lib.stride_tricks.as_strided(
                sub, shape=(NG, 128, XC), strides=(RV * s0, s0, s1))
            xt[:, :, part, :] = win.transpose(1, 0, 2)
        in_maps.append({"xs": xt.reshape(128, -1), "bmat": bmat,
                        "biast": bias_col})

    res = bass_utils.run_bass_kernel_spmd(nc, in_maps,
                                          core_ids=list(range(NCORES)),
                                          trace=TRACE)
    global LAST_EXEC_NS
    LAST_EXEC_NS = res.exec_time_ns

    cols = []
    for c in range(NCORES):
        arr = np.asarray(res.results[c]["out"]).astype(np.float32)
        v = arr.reshape(128, NG, COLS).transpose(1, 0, 2)[:, :RV, :]
        cols.append(v.reshape(OH, COLS))
    out = np.hstack(cols)
    return np.ascontiguousarray(out[:, :OW])


# revision 14
# speedup vs baseline: 1.3311x; 1.3311x over previous
"""Trainium2 Bass kernel: 4096x4096 valid 5x5 cross-correlation + scalar bias.

Strategy (8 NeuronCores, SPMD):
  - Shard the OUTPUT by columns: core c computes out[:, 512c : 512c+512]
    (core 7's last 4 columns are padding, trimmed after gather). Each core
    reads x rows 0..4095, cols [512c, 512c+516) (host-padded to width 4100).
  - On-core: the 5x5 conv is computed as banded-matrix matmuls on the
    TensorEngine. For an input row-tile X_g = x[124g : 124g+128, :] and
    kernel column dj, the banded matrix B_dj[k, m] = w[k-m, dj] gives
      (B_dj^T @ X_g[:, dj:dj+512])[m, n] = sum_di w[di, dj] x[124g+m+di, n+dj]
    so accumulating the 5 dj-matmuls in PSUM yields 124 valid output rows
    per tile. 4092 = 33 * 124 exactly; 33 tiles cover rows 0..4095 exactly.
  - All matmul operands are bf16 (rel err ~2.7e-3, well under the 2e-2
    gate). This halves input DMA bytes vs f32 and halves LDWEIGHTS time.
  - Input is pre-tiled on the host into partition-major layout
    (128, 33*516): partition p, tile g holds x row 124g+p. Input then
    streams as 8 large contiguous-per-partition DMAs (chunks of 1..6
    tiles) instead of 33 separate row-window DMAs: fewer descriptor
    pushes (the per-push DGE cost is ~0.6us) and full 16-engine fan-out.
  - Output is staged in SBUF as bf16 (PSUM->SBUF drain fuses the bias add
    and the f32->bf16 cast, split across DVE/ACT/Pool engines) and leaves
    the device partition-major as (128, 33*512) bf16 in 9 per-block DMAs.
    The host transposes back to row-major and upcasts to f32. bf16 output
    halves the output DMA bytes.
"""
import os

os.environ.setdefault("MYCRO_LOCAL_CACHE", "1")

import numpy as np
import ml_dtypes

import concourse.bass as bass
import concourse.bacc as bacc
import concourse.tile as tile
import concourse.mybir as mybir
from concourse import bass_utils

H, W = 4096, 4096
KH, KW = 5, 5
OH, OW = H - KH + 1, W - KW + 1          # 4092, 4092
NCORES = 8
COLS = 512                               # output cols per core
XC = COLS + KW - 1                       # 516 input cols per core
NG = 33                                  # row tiles per core (33*124 = 4092)
RV = 124                                 # valid output rows per tile
BLK = 8                                  # tiles per output-DMA block
# input stream chunk sizes (tiles per DMA): small first chunk so the first
# matmul starts early, larger after to amortize descriptor-push cost
CHUNKS = [1, 2, 3, 4, 5, 6, 6, 6]
assert sum(CHUNKS) == NG

_compiled = None
TRACE = False            # test harness can flip this for neuron-profile timing
LAST_EXEC_NS = None

BF16 = mybir.dt.bfloat16


def _build():
    nc = bacc.Bacc("TRN2", target_bir_lowering=False, debug=False,
                   num_devices=NCORES)

    x_dram = nc.dram_tensor("xs", (128, NG * XC), BF16, kind="ExternalInput")
    b_dram = nc.dram_tensor("bmat", (128, KW * 128), BF16,
                            kind="ExternalInput")
    bias_dram = nc.dram_tensor("biast", (128, 1), mybir.dt.float32,
                               kind="ExternalInput")
    out_dram = nc.dram_tensor("out", (128, NG * COLS), BF16,
                              kind="ExternalOutput")

    # chunk start tile index
    cstart = [0]
    for c in CHUNKS:
        cstart.append(cstart[-1] + c)
    blocks = [list(range(s, min(s + BLK, NG))) for s in range(0, NG, BLK)]

    with tile.TileContext(nc) as tc:
        with (
            tc.tile_pool(name="const", bufs=1) as cpool,
            tc.tile_pool(name="x", bufs=1) as xpool,
            tc.tile_pool(name="stage", bufs=3) as spool,
            tc.tile_pool(name="psum", bufs=8, space=bass.MemorySpace.PSUM) as ppool,
        ):
            bt = cpool.tile([128, KW * 128], BF16)
            biast = cpool.tile([128, 1], mybir.dt.float32)

            # input chunks: chunk ci holds tiles [cstart[ci], cstart[ci+1]).
            # Interleave pushes across the two HWDGE rings (sync + scalar)
            # so the first chunks land as early as possible.
            xts = [xpool.tile([128, csz * XC], BF16, name=f"xc{ci}",
                              tag=f"xc{ci}")
                   for ci, csz in enumerate(CHUNKS)]

            def push_chunk(ci):
                eng = nc.sync if ci % 2 == 0 else nc.scalar
                eng.dma_start(
                    xts[ci][:],
                    x_dram.ap()[:, cstart[ci] * XC:cstart[ci + 1] * XC])

            # bt gates the first matmul: first on the sync ring
            nc.sync.dma_start(bt[:], b_dram.ap())
            push_chunk(1)   # scalar ring: chunk 1 first
            push_chunk(0)   # sync ring: chunk 0 right after bt
            push_chunk(2)
            push_chunk(3)
            nc.scalar.dma_start(biast[:], bias_dram.ap())
            for ci in range(4, len(CHUNKS)):
                push_chunk(ci)

            g2chunk = []
            for ci, csz in enumerate(CHUNKS):
                g2chunk += [ci] * csz

            # gpsimd cannot read PSUM; alternate DVE / ACT for the drains
            drain_eng = lambda lg: [nc.vector, nc.scalar,
                                    nc.vector, nc.scalar][lg % 4]
            # gpsimd's SWDGE queue is slow (~1.7us for 524KB); keep the two
            # tail blocks on the fast HWDGE rings so the kernel ends promptly
            out_ring = lambda bi: [nc.gpsimd, nc.sync, nc.gpsimd][bi % 3] \
                if bi < len(blocks) - 2 else (nc.scalar, nc.sync)[len(blocks) - 1 - bi]

            for bi, blk in enumerate(blocks):
                stg = spool.tile([128, len(blk) * COLS], BF16,
                                 name=f"stg{bi}", tag="stg")
                for lg, g in enumerate(blk):
                    ci = g2chunk[g]
                    off = (g - cstart[ci]) * XC
                    xt = xts[ci]
                    ps = ppool.tile([128, COLS], mybir.dt.float32,
                                    name=f"ps{g}", tag="ps")
                    for dj in range(KW):
                        nc.tensor.matmul(
                            ps[:],
                            bt[:, dj * 128:(dj + 1) * 128],
                            xt[:, off + dj:off + dj + COLS],
                            start=(dj == 0),
                            stop=(dj == KW - 1),
                        )
                    # drain PSUM -> stage (bias add + f32->bf16 cast); all
                    # 128 rows so the staged tile is fully defined (rows
                    # 124..127 are partial sums the host discards)
                    dst = stg[:, lg * COLS:(lg + 1) * COLS]
                    if g == NG - 1:
                        # final tile: split the drain across both engines so
                        # the critical tail (drain -> last DMA -> done) is
                        # as short as possible
                        nc.vector.tensor_scalar_add(
                            dst[:, :COLS // 2], ps[:, :COLS // 2], biast[:])
                        nc.scalar.activation(
                            dst[:, COLS // 2:], ps[:, COLS // 2:],
                            mybir.ActivationFunctionType.Identity,
                            bias=biast[:])
                    elif drain_eng(lg) is nc.scalar:
                        nc.scalar.activation(
                            dst, ps[:],
                            mybir.ActivationFunctionType.Identity,
                            bias=biast[:])
                    else:
                        nc.vector.tensor_scalar_add(dst, ps[:], biast[:])
                # one output DMA per block: contiguous partition-major span
                out_ring(bi).dma_start(
                    out_dram.ap()[:, blk[0] * COLS:(blk[-1] + 1) * COLS],
                    stg[:, :len(blk) * COLS])

    nc.compile()
    return nc


def _banded(weight: np.ndarray) -> np.ndarray:
    ball = np.zeros((128, KW * 128), dtype=np.float32)
    for dj in range(KW):
        for di in range(KH):
            m = np.arange(128 - di)
            ball[m + di, dj * 128 + m] = weight[di, dj]
    return ball


def kernel(x: np.ndarray, weight: np.ndarray, bias: np.ndarray) -> np.ndarray:
    global _compiled
    x = np.ascontiguousarray(np.asarray(x, dtype=np.float32))
    weight = np.asarray(weight, dtype=np.float32)
    bias = np.asarray(bias, dtype=np.float32)

    if _compiled is None:
        _compiled = _build()
    nc = _compiled

    xpad = np.zeros((H, NCORES * COLS + KW - 1), dtype=np.float32)
    xpad[:, :W] = x
    xpad = xpad.astype(ml_dtypes.bfloat16)
    ball = _banded(weight).astype(ml_dtypes.bfloat16)
    bias_col = np.full((128, 1), bias[0], dtype=np.float32)

    in_maps = []
    s0, s1 = xpad.strides
    for c in range(NCORES):
        sub = xpad[:, COLS * c: COLS * c + XC]
        win = np.lib.stride_tricks.as_strided(
            sub, shape=(NG, 128, XC), strides=(RV * s0, s0, s1))
        xt = np.ascontiguousarray(win.transpose(1, 0, 2)).reshape(128, -1)
        in_maps.append({"xs": xt, "bmat": ball, "biast": bias_col})

    res = bass_utils.run_bass_kernel_spmd(nc, in_maps,
                                          core_ids=list(range(NCORES)),
                                          trace=TRACE)
    global LAST_EXEC_NS
    LAST_EXEC_NS = res.exec_time_ns

    cols = []
    for c in range(NCORES):
        arr = np.asarray(res.results[c]["out"]).astype(np.float32)
        v = arr.reshape(128, NG, COLS).transpose(1, 0, 2)[:, :RV, :]
        cols.append(v.reshape(OH, COLS))
    out = np.hstack(cols)
    return np.ascontiguousarray(out[:, :OW])
